# revision 12
# baseline (speedup 1.0000x reference)
"""Fused cross-attention kernel for Trainium2, 8-way data-parallel over batch.

Per core (one batch element), all big matmuls in fp8e4m3 DoubleRow mode
(2 contraction chunks of 128 per instruction, 0.5 cycles/row = 4x the f32r
rate):

  QT[d, hw] = (Wq @ Jp + bq)   f32r matmuls (bias folded via ones row),
  K [d, hw] = (Wk @ Jg + bk)   PSUM -> SBUF fp8e4m3 copies on DVE/Act
  V [hw, d] = (Jg.T @ WvT + bv)  stored as [V | 1 | 1] (denominator cols)
  For each q-block (512 queries), for each key-chunk pair (2 x 128 keys):
    S^T[k, 2, q] = K-pair^T @ QT     one 2-bank PSUM tile, 2 DR matmuls
    E^T = exp(S^T/16) -> fp8        Act engine: exact exp activation
                                     DVE pairs: Schraudolph bits via
                                     tensor_scalar f32->u8 (RNE convert)
    O[q, 258] += E^T-subtile.T @ [V|1|1]   DR matmuls, col 256 = sum E
  out[q, d] = O[:, :256] * (1 / O[:, 256])

The exp work (16.8M elements/core) is the bottleneck; it is split between
the Activation engine (exact exp, 1.2 GHz) and DVE (Schraudolph-to-fp8-bits
affine, 0.96 GHz). GPSIMD cannot read PSUM on TRN2, so it cannot help.
"""

import dataclasses
import sys

sys.path.insert(0, "/opt/trn_rl_repo")

import numpy as np

import concourse.bacc as bacc
import concourse.mybir as mybir
import concourse.tile as tile
from concourse.bass_utils import run_bass_kernel_spmd

B, C, H, W = 8, 64, 64, 64
HW = H * W  # 4096
D = 256
CE = C + 1  # channels + ones row for bias folding
N_CORES = 8
QB = 512  # queries per block
N_QB = HW // QB  # 8
N_KC = HW // 128  # 32 key chunks
NPAIR = N_KC // 2  # 16 key-chunk pairs
DV = D + 2  # V row width: 256 values + 2 ones columns
F32 = mybir.dt.float32
F32R = mybir.dt.float32r
F8 = mybir.dt.float8e4
U8 = mybir.dt.uint8
DR = mybir.MatmulPerfMode.DoubleRow
SWI = mybir.MatmulPerfMode.DoubleRowSwInterleave

# Schraudolph exp -> e4m3 bits: bits = rne(x * 8*log2(e)/16 + (56 - 0.344))
A_MUL = 8.0 * float(np.log2(np.e)) / 16.0
B_ADD = 56.0 - 0.344

_CACHE = {}


def build_module(
    reps: int = 1,
    st_bufs: int = 2,
    op_bufs: int = 4,
    ep_bufs: int = 4,
    pp_bufs: int = 4,
    dve_pairs: tuple = (2, 5, 8, 11, 14),
    act_copies: bool = False,
    act_scales: bool = False,
):
    nc = bacc.Bacc("TRN2", target_bir_lowering=False)
    jp_d = nc.dram_tensor("jp", [CE, HW], F32R, kind="ExternalInput")
    jg_d = nc.dram_tensor("jg", [CE, HW], F32R, kind="ExternalInput")
    wq_d = nc.dram_tensor("wq", [CE, D], F32R, kind="ExternalInput")
    wk_d = nc.dram_tensor("wk", [CE, D], F32R, kind="ExternalInput")
    wv_d = nc.dram_tensor("wv", [CE, D], F32R, kind="ExternalInput")
    ones_d = nc.dram_tensor("ones", [128, N_KC, 2], F8, kind="ExternalInput")
    onesz_d = nc.dram_tensor("onesz", [128, 256], F8, kind="ExternalInput")
    out_d = nc.dram_tensor("out", [D, HW], F32, kind="ExternalOutput")

    dve_set = set(dve_pairs)

    with tile.TileContext(nc) as tc:
        with tc.tile_pool(name="const", bufs=1) as const:
            jp_t = const.tile([CE, HW], F32R, tag="jp")
            jg_t = const.tile([CE, HW], F32R, tag="jg")
            wq_t = const.tile([CE, D], F32R, tag="wq")
            wk_t = const.tile([CE, D], F32R, tag="wk")
            wv_t = const.tile([CE, D], F32R, tag="wv")
            qt_b = [
                const.tile([128, 2, QB], F8, tag=f"qt{g}", name=f"qt_{g}")
                for g in range(N_QB)
            ]
            # K tiles in SwInterleave layout: [128, 4 chunks, 256] where
            # column 2*(127-q)+dh of chunk j holds K[dh, 128j+q]
            kt_g = [
                const.tile([128, 4, 256], F8, tag=f"kt{g}", name=f"kt_{g}")
                for g in range(N_QB)
            ]
            # V tiles as SwInterleave weights: [128, 2 pairs, 2 d-halves,
            # 256] where col 2*(127-d%128)+kc of (pair, half) = V[kc, d]
            vt_g = [
                const.tile([128, 2, 3, 256], F8, tag=f"vt{g}", name=f"vt_{g}")
                for g in range(N_QB)
            ]

            nc.sync.dma_start(wq_t[:], wq_d[:])
            nc.sync.dma_start(wk_t[:], wk_d[:])
            nc.sync.dma_start(wv_t[:], wv_d[:])
            for g in range(N_QB):
                for pr in range(2):
                    nc.sync.dma_start(vt_g[g][:, pr, 2, :], onesz_d[:])
                hs = slice(g * QB, (g + 1) * QB)
                nc.sync.dma_start(jg_t[:, hs], jg_d[:, hs])
                nc.sync.dma_start(jp_t[:, hs], jp_d[:, hs])

            def copy_f8(eng_is_act, dst, src):
                if eng_is_act and act_copies:
                    nc.scalar.activation(
                        dst, src, mybir.ActivationFunctionType.Copy
                    )
                else:
                    nc.vector.tensor_copy(dst, src)

            for _rep in range(reps):
                # ---- projections (f32r) ----
                with tc.tile_pool(name="pp", bufs=pp_bufs, space="PSUM") as pp:

                    def proj_q(g):
                        hs = slice(g * QB, (g + 1) * QB)
                        for dh in range(2):
                            ds = slice(dh * 128, (dh + 1) * 128)
                            psq = pp.tile([128, QB], F32, tag="proj")
                            nc.tensor.matmul(psq[:], wq_t[:, ds], jp_t[:, hs])
                            copy_f8(dh == 1, qt_b[g][:, dh, :], psq[:])

                    proj_q(0)
                    for g in range(N_QB):
                        hs = slice(g * QB, (g + 1) * QB)
                        for dh in range(2):
                            ds = slice(dh * 128, (dh + 1) * 128)
                            psk = pp.tile([128, QB], F32, tag="proj")
                            nc.tensor.matmul(psk[:], wk_t[:, ds], jg_t[:, hs])
                            dst = dataclasses.replace(
                                kt_g[g][:],
                                ap=[[1024, 128], [256, 4], [-2, 128]],
                                offset=kt_g[g][:].offset + 254 + dh,
                            )
                            nc.vector.tensor_copy(dst, psk[:])
                        for j in range(4):
                            ck = 4 * g + j
                            ks = slice(ck * 128, (ck + 1) * 128)
                            psv = pp.tile([128, D], F32, tag="projv")
                            nc.tensor.matmul(psv[:], jg_t[:, ks], wv_t[:])
                            pair, kc = j // 2, j % 2
                            dst = dataclasses.replace(
                                vt_g[g][:],
                                ap=[[1536, 128], [256, 2], [-2, 128]],
                                offset=vt_g[g][:].offset + pair * 768 + 254 + kc,
                            )
                            nc.vector.tensor_copy(dst, psv[:])
                    for g in range(1, N_QB):
                        proj_q(g)

                # ---- attention ----
                with (
                    tc.tile_pool(name="stp", bufs=st_bufs, space="PSUM") as stp,
                    tc.tile_pool(name="op", bufs=3, space="PSUM") as op,
                    tc.tile_pool(name="dnp", bufs=1, space="PSUM") as dnp,
                    tc.tile_pool(name="ep", bufs=ep_bufs) as ep,
                    tc.tile_pool(name="outp", bufs=3) as outp,
                    tc.tile_pool(name="lp", bufs=4) as lp,
                ):
                    for qb in range(N_QB):
                        o_ps = [
                            op.tile([128, QB], F32, tag="o", name=f"o_{qb}_{h}")
                            for h in range(2)
                        ]
                        den_ps = dnp.tile(
                            [128, QB], F32, tag="dn", name=f"dn_{qb}"
                        )
                        st_t = [None] * NPAIR
                        et_t = [None] * NPAIR

                        def s_pair(p):
                            st2 = stp.tile([128, 2, QB], F32, tag="st")
                            st_t[p] = st2
                            for c in range(2):
                                ck = 2 * p + c
                                g, j = ck // 4, ck % 4
                                nc.tensor.matmul(
                                    st2[:, c, :],
                                    kt_g[g][:, j, :],
                                    qt_b[qb][:],
                                    start=True,
                                    stop=True,
                                    perf_mode=SWI,
                                )

                        def exp_pair(p):
                            et2 = ep.tile([128, 2, QB], F8, tag="e")
                            et_t[p] = et2
                            if p in dve_set:
                                nc.vector.tensor_scalar(
                                    et2[:].bitcast(U8),
                                    st_t[p][:, :, :],
                                    A_MUL,
                                    B_ADD,
                                    mybir.AluOpType.mult,
                                    mybir.AluOpType.add,
                                )
                            else:
                                nc.scalar.activation(
                                    et2[:, :, :],
                                    st_t[p][:, :, :],
                                    mybir.ActivationFunctionType.Exp,
                                    scale=1.0 / 16.0,
                                )

                        def pv_pair(p):
                            g, pr = p // 2, p % 2
                            et2 = et_t[p]
                            for h in range(2):
                                nc.tensor.matmul(
                                    o_ps[h][:],
                                    vt_g[g][:, pr, h, :],
                                    et2[:],
                                    start=(p == 0),
                                    stop=(p == NPAIR - 1),
                                    perf_mode=SWI,
                                )
                            nc.tensor.matmul(
                                den_ps[:],
                                vt_g[g][:, pr, 2, :],
                                et2[:],
                                start=(p == 0),
                                stop=(p == NPAIR - 1),
                                perf_mode=SWI,
                            )

                        # software pipeline: issue S(p+1) before PV(p) so the
                        # PE never sits behind exp(p) with S work available
                        s_pair(0)
                        for p in range(NPAIR):
                            exp_pair(p)
                            if p + 1 < NPAIR:
                                s_pair(p + 1)
                            pv_pair(p)

                        linv = lp.tile([1, QB], F32, tag="l")
                        nc.vector.reciprocal(linv[:], den_ps[0:1, :])
                        lbc = lp.tile([128, QB], F32, tag="lb")
                        nc.gpsimd.partition_broadcast(lbc[:], linv[:])
                        qs = slice(qb * QB, (qb + 1) * QB)
                        for h in range(2):
                            ot = outp.tile([128, QB], F32, tag="ot")
                            nc.vector.tensor_tensor(
                                ot[:], o_ps[h][:], lbc[:], mybir.AluOpType.mult
                            )
                            nc.sync.dma_start(
                                out_d[h * 128 : (h + 1) * 128, qs], ot[:]
                            )

    nc.compile()
    return nc


def _get_module(reps: int = 1, **kw):
    key = (reps, tuple(sorted(kw.items())))
    if key not in _CACHE:
        _CACHE[key] = build_module(reps, **kw)
    return _CACHE[key]


_ROW1 = np.ones((1, HW), np.float32)


def _prep_in_maps(inputs, **_):
    import ml_dtypes

    f8 = ml_dtypes.float8_e4m3
    jp = np.asarray(inputs["Jp_embedding"], np.float32).reshape(B, C, HW)
    jg = np.asarray(inputs["Jg_embedding"], np.float32).reshape(B, C, HW)
    wq = np.concatenate(
        [
            np.asarray(inputs["Wq"], np.float32).T,
            np.asarray(inputs["bq"], np.float32)[None, :],
        ],
        0,
    )
    wk = np.concatenate(
        [
            np.asarray(inputs["Wk"], np.float32).T,
            np.asarray(inputs["bk"], np.float32)[None, :],
        ],
        0,
    )
    wv = np.concatenate(
        [
            np.asarray(inputs["Wv"], np.float32).T,
            np.asarray(inputs["bv"], np.float32)[None, :],
        ],
        0,
    )
    ones = np.ones((128, N_KC, 2), f8)
    onesz = np.zeros((128, 256), np.float32)
    onesz[:, 254:256] = 1.0
    onesz = onesz.astype(f8)
    return [
        {
            "jp": np.concatenate([jp[b], _ROW1], 0),
            "jg": np.concatenate([jg[b], _ROW1], 0),
            "wq": wq,
            "wk": wk,
            "wv": wv,
            "ones": ones,
            "onesz": onesz,
        }
        for b in range(B)
    ]


def kernel(**inputs):
    nc = _get_module()
    in_maps = _prep_in_maps(inputs)
    # The first execution of a freshly loaded NEFF can race DMA/engine
    # warm-up (observed on HW; CoreSim-clean). Run once to warm up, then
    # return the second execution's results.
    run_bass_kernel_spmd(nc, in_maps, core_ids=list(range(N_CORES)))
    res = run_bass_kernel_spmd(nc, in_maps, core_ids=list(range(N_CORES)))
    return np.stack(
        [
            np.ascontiguousarray(res.results[b]["out"].T).reshape(D, H, W)
            for b in range(B)
        ],
        axis=0,
    )


# revision 13
# speedup vs baseline: 1.5028x; 1.5028x over previous
"""Fused cross-attention kernel for Trainium2, 8-way data-parallel over batch.

Per core (one batch element), all big matmuls in fp8e4m3 DoubleRow mode
(2 contraction chunks of 128 per instruction, 0.5 cycles/row = 4x the f32r
rate):

  QT[d, hw] = (Wq @ Jp + bq)   f32r matmuls (bias folded via ones row),
  K [d, hw] = (Wk @ Jg + bk)   PSUM -> SBUF fp8e4m3 copies on DVE/Act
  V [hw, d] = (Jg.T @ WvT + bv)  stored as [V | 1 | 1] (denominator cols)
  For each q-block (512 queries), for each key-chunk pair (2 x 128 keys):
    S^T[k, 2, q] = K-pair^T @ QT     one 2-bank PSUM tile, 2 DR matmuls
    E^T = exp(S^T/16) -> fp8        Act engine: exact exp activation
                                     DVE pairs: Schraudolph bits via
                                     tensor_scalar f32->u8 (RNE convert)
    O[q, 258] += E^T-subtile.T @ [V|1|1]   DR matmuls, col 256 = sum E
  out[q, d] = O[:, :256] * (1 / O[:, 256])

The exp work (16.8M elements/core) is the bottleneck; it is split between
the Activation engine (exact exp, 1.2 GHz) and DVE (Schraudolph-to-fp8-bits
affine, 0.96 GHz). GPSIMD cannot read PSUM on TRN2, so it cannot help.
"""

import dataclasses
import sys

sys.path.insert(0, "/opt/trn_rl_repo")

import numpy as np

import concourse.bacc as bacc
import concourse.mybir as mybir
import concourse.tile as tile
from concourse.bass_utils import run_bass_kernel_spmd

B, C, H, W = 8, 64, 64, 64
HW = H * W  # 4096
D = 256
CE = C + 1  # channels + ones row for bias folding
N_CORES = 8
QB = 512  # queries per block
N_QB = HW // QB  # 8
N_KC = HW // 128  # 32 key chunks
NPAIR = N_KC // 2  # 16 key-chunk pairs
DV = D + 2  # V row width: 256 values + 2 ones columns
F32 = mybir.dt.float32
F32R = mybir.dt.float32r
F8 = mybir.dt.float8e4
U8 = mybir.dt.uint8
DR = mybir.MatmulPerfMode.DoubleRow
SWI = mybir.MatmulPerfMode.DoubleRowSwInterleave

# Schraudolph exp -> e4m3 bits: bits = rne(x * 8*log2(e)/16 + (56 - 0.344))
A_MUL = 8.0 * float(np.log2(np.e)) / 16.0
B_ADD = 56.0 - 0.344

_CACHE = {}


def build_module(
    reps: int = 1,
    st_bufs: int = 2,
    op_bufs: int = 4,
    ep_bufs: int = 4,
    pp_bufs: int = 4,
    dve_pairs: tuple = (1, 4, 6, 9, 11, 14),
    act_copies: bool = False,
    act_scales: bool = False,
):
    nc = bacc.Bacc("TRN2", target_bir_lowering=False)
    jp_d = nc.dram_tensor("jp", [CE, HW], F32R, kind="ExternalInput")
    jg_d = nc.dram_tensor("jg", [CE, HW], F32R, kind="ExternalInput")
    wq_d = nc.dram_tensor("wq", [CE, D], F32R, kind="ExternalInput")
    wk_d = nc.dram_tensor("wk", [CE, D], F32R, kind="ExternalInput")
    wv_d = nc.dram_tensor("wv", [CE, D], F32R, kind="ExternalInput")
    ones_d = nc.dram_tensor("ones", [128, N_KC, 2], F8, kind="ExternalInput")
    out_d = nc.dram_tensor("out", [HW, D], F32, kind="ExternalOutput")

    dve_set = set(dve_pairs)

    with tile.TileContext(nc) as tc:
        with tc.tile_pool(name="const", bufs=1) as const:
            jp_t = const.tile([CE, HW], F32R, tag="jp")
            jg_t = const.tile([CE, HW], F32R, tag="jg")
            wq_t = const.tile([CE, D], F32R, tag="wq")
            wk_t = const.tile([CE, D], F32R, tag="wk")
            wv_t = const.tile([CE, D], F32R, tag="wv")
            qt_b = [
                const.tile([128, 2, QB], F8, tag=f"qt{g}", name=f"qt_{g}")
                for g in range(N_QB)
            ]
            # K tiles in SwInterleave layout: [128, 4 chunks, 256] where
            # column 2*(127-q)+dh of chunk j holds K[dh, 128j+q]
            kt_g = [
                const.tile([128, 4, 256], F8, tag=f"kt{g}", name=f"kt_{g}")
                for g in range(N_QB)
            ]
            vt_g = [
                const.tile([128, 4, DV], F8, tag=f"vt{g}", name=f"vt_{g}")
                for g in range(N_QB)
            ]

            nc.sync.dma_start(wq_t[:], wq_d[:])
            nc.sync.dma_start(wk_t[:], wk_d[:])
            nc.sync.dma_start(wv_t[:], wv_d[:])
            for g in range(N_QB):
                hs = slice(g * QB, (g + 1) * QB)
                nc.sync.dma_start(jg_t[:, hs], jg_d[:, hs])
                nc.sync.dma_start(jp_t[:, hs], jp_d[:, hs])
                nc.sync.dma_start(vt_g[g][:, :, D:DV], ones_d[:, 4 * g : 4 * g + 4, :])

            def copy_f8(eng_is_act, dst, src):
                if eng_is_act and act_copies:
                    nc.scalar.activation(
                        dst, src, mybir.ActivationFunctionType.Copy
                    )
                else:
                    nc.vector.tensor_copy(dst, src)

            for _rep in range(reps):
                # ---- projections (f32r) ----
                with tc.tile_pool(name="pp", bufs=pp_bufs, space="PSUM") as pp:

                    def proj_q(g):
                        hs = slice(g * QB, (g + 1) * QB)
                        for dh in range(2):
                            ds = slice(dh * 128, (dh + 1) * 128)
                            psq = pp.tile([128, QB], F32, tag="proj")
                            nc.tensor.matmul(psq[:], wq_t[:, ds], jp_t[:, hs])
                            copy_f8(dh == 1, qt_b[g][:, dh, :], psq[:])

                    proj_q(0)
                    for g in range(N_QB):
                        hs = slice(g * QB, (g + 1) * QB)
                        for dh in range(2):
                            ds = slice(dh * 128, (dh + 1) * 128)
                            psk = pp.tile([128, QB], F32, tag="proj")
                            nc.tensor.matmul(psk[:], wk_t[:, ds], jg_t[:, hs])
                            dst = dataclasses.replace(
                                kt_g[g][:],
                                ap=[[1024, 128], [256, 4], [-2, 128]],
                                offset=kt_g[g][:].offset + 254 + dh,
                            )
                            nc.vector.tensor_copy(dst, psk[:])
                        for j in range(4):
                            ck = 4 * g + j
                            ks = slice(ck * 128, (ck + 1) * 128)
                            psv = pp.tile([128, D], F32, tag="projv")
                            nc.tensor.matmul(psv[:], jg_t[:, ks], wv_t[:])
                            copy_f8(j % 2 == 0, vt_g[g][:, j, :D], psv[:])
                    for g in range(1, N_QB):
                        proj_q(g)

                # ---- attention ----
                with (
                    tc.tile_pool(name="stp", bufs=st_bufs, space="PSUM") as stp,
                    tc.tile_pool(name="op", bufs=op_bufs, space="PSUM") as op,
                    tc.tile_pool(name="ep", bufs=ep_bufs) as ep,
                    tc.tile_pool(name="outp", bufs=3) as outp,
                    tc.tile_pool(name="lp", bufs=4) as lp,
                ):
                    for qb in range(N_QB):
                        o_ps = [
                            op.tile([128, DV], F32, tag="o", name=f"o_{qb}_{i}")
                            for i in range(4)
                        ]
                        st_t = [None] * NPAIR
                        et_t = [None] * NPAIR

                        def s_pair(p):
                            st2 = stp.tile([128, 2, QB], F32, tag="st")
                            st_t[p] = st2
                            for c in range(2):
                                ck = 2 * p + c
                                g, j = ck // 4, ck % 4
                                nc.tensor.matmul(
                                    st2[:, c, :],
                                    kt_g[g][:, j, :],
                                    qt_b[qb][:],
                                    start=True,
                                    stop=True,
                                    perf_mode=SWI,
                                )

                        def exp_pair(p):
                            et2 = ep.tile([128, 2, QB], F8, tag="e")
                            et_t[p] = et2
                            if p in dve_set:
                                nc.vector.tensor_scalar(
                                    et2[:].bitcast(U8),
                                    st_t[p][:, :, :],
                                    A_MUL,
                                    B_ADD,
                                    mybir.AluOpType.mult,
                                    mybir.AluOpType.add,
                                )
                            else:
                                nc.scalar.activation(
                                    et2[:, :, :],
                                    st_t[p][:, :, :],
                                    mybir.ActivationFunctionType.Exp,
                                    scale=1.0 / 16.0,
                                )

                        def pv_pair(p):
                            g, jj = p // 2, 2 * (p % 2)
                            et2 = et_t[p]
                            for i in range(4):
                                nc.tensor.matmul(
                                    o_ps[i][:],
                                    et2[:, :, i * 128 : (i + 1) * 128],
                                    vt_g[g][:, jj : jj + 2, :],
                                    start=(p == 0),
                                    stop=(p == NPAIR - 1),
                                    perf_mode=DR,
                                )

                        # software pipeline: issue S(p+1) before PV(p) so the
                        # PE never sits behind exp(p) with S work available
                        s_pair(0)
                        for p in range(NPAIR):
                            exp_pair(p)
                            if p + 1 < NPAIR:
                                s_pair(p + 1)
                            pv_pair(p)

                        for qsub in range(4):
                            row = qb * 4 + qsub
                            linv = lp.tile([128, 1], F32, tag="l")
                            nc.vector.reciprocal(linv[:], o_ps[qsub][:, D : D + 1])
                            ot = outp.tile([128, D], F32, tag="ot")
                            if qsub % 2 == 0 and act_scales:
                                nc.scalar.activation(
                                    ot[:],
                                    o_ps[qsub][:, :D],
                                    mybir.ActivationFunctionType.Copy,
                                    scale=linv[:],
                                )
                            else:
                                nc.vector.tensor_scalar_mul(
                                    ot[:], o_ps[qsub][:, :D], linv[:]
                                )
                            nc.sync.dma_start(
                                out_d[row * 128 : (row + 1) * 128, :], ot[:]
                            )

    nc.compile()
    return nc


def _get_module(reps: int = 1, **kw):
    key = (reps, tuple(sorted(kw.items())))
    if key not in _CACHE:
        _CACHE[key] = build_module(reps, **kw)
    return _CACHE[key]


_ROW1 = np.ones((1, HW), np.float32)


def _prep_in_maps(inputs, **_):
    import ml_dtypes

    f8 = ml_dtypes.float8_e4m3
    jp = np.asarray(inputs["Jp_embedding"], np.float32).reshape(B, C, HW)
    jg = np.asarray(inputs["Jg_embedding"], np.float32).reshape(B, C, HW)
    wq = np.concatenate(
        [
            np.asarray(inputs["Wq"], np.float32).T,
            np.asarray(inputs["bq"], np.float32)[None, :],
        ],
        0,
    )
    wk = np.concatenate(
        [
            np.asarray(inputs["Wk"], np.float32).T,
            np.asarray(inputs["bk"], np.float32)[None, :],
        ],
        0,
    )
    wv = np.concatenate(
        [
            np.asarray(inputs["Wv"], np.float32).T,
            np.asarray(inputs["bv"], np.float32)[None, :],
        ],
        0,
    )
    ones = np.ones((128, N_KC, 2), f8)
    return [
        {
            "jp": np.concatenate([jp[b], _ROW1], 0),
            "jg": np.concatenate([jg[b], _ROW1], 0),
            "wq": wq,
            "wk": wk,
            "wv": wv,
            "ones": ones,
        }
        for b in range(B)
    ]


def kernel(**inputs):
    nc = _get_module()
    in_maps = _prep_in_maps(inputs)
    # The first execution of a freshly loaded NEFF can race DMA/engine
    # warm-up (observed on HW; CoreSim-clean). Run once to warm up, then
    # return the second execution's results.
    run_bass_kernel_spmd(nc, in_maps, core_ids=list(range(N_CORES)))
    res = run_bass_kernel_spmd(nc, in_maps, core_ids=list(range(N_CORES)))
    return np.stack(
        [res.results[b]["out"].reshape(D, H, W) for b in range(B)], axis=0
    )


# revision 14
# speedup vs baseline: 1.7978x; 1.1963x over previous
"""Fused cross-attention kernel for Trainium2, 8-way data-parallel over batch.

Per core (one batch element), all big matmuls in fp8e4m3 DoubleRow mode
(2 contraction chunks of 128 per instruction, 0.5 cycles/row = 4x the f32r
rate):

  QT[d, hw] = (Wq @ Jp + bq)   f32r matmuls (bias folded via ones row),
  K [d, hw] = (Wk @ Jg + bk)   PSUM -> SBUF fp8e4m3 copies on DVE/Act
  V [hw, d] = (Jg.T @ WvT + bv)  stored as [V | 1 | 1] (denominator cols)
  For each q-block (512 queries), for each key-chunk pair (2 x 128 keys):
    S^T[k, 2, q] = K-pair^T @ QT     one 2-bank PSUM tile, 2 DR matmuls
    E^T = exp(S^T/16) -> fp8        Act engine: exact exp activation
                                     DVE pairs: Schraudolph bits via
                                     tensor_scalar f32->u8 (RNE convert)
    O[q, 258] += E^T-subtile.T @ [V|1|1]   DR matmuls, col 256 = sum E
  out[q, d] = O[:, :256] * (1 / O[:, 256])

The exp work (16.8M elements/core) is the bottleneck; it is split between
the Activation engine (exact exp, 1.2 GHz) and DVE (Schraudolph-to-fp8-bits
affine, 0.96 GHz). GPSIMD cannot read PSUM on TRN2, so it cannot help.
"""

import dataclasses
import sys

sys.path.insert(0, "/opt/trn_rl_repo")

import numpy as np

import concourse.bacc as bacc
import concourse.mybir as mybir
import concourse.tile as tile
from concourse.bass_utils import run_bass_kernel_spmd

B, C, H, W = 8, 64, 64, 64
HW = H * W  # 4096
D = 256
CE = C + 1  # channels + ones row for bias folding
N_CORES = 8
QB = 512  # queries per block
N_QB = HW // QB  # 8
N_KC = HW // 128  # 32 key chunks
NPAIR = N_KC // 2  # 16 key-chunk pairs
DV = D + 2  # V row width: 256 values + 2 ones columns
F32 = mybir.dt.float32
F32R = mybir.dt.float32r
F8 = mybir.dt.float8e4
U8 = mybir.dt.uint8
DR = mybir.MatmulPerfMode.DoubleRow
SWI = mybir.MatmulPerfMode.DoubleRowSwInterleave

# Schraudolph exp -> e4m3 bits: bits = rne(x * 8*log2(e)/16 + (56 - 0.344))
A_MUL = 8.0 * float(np.log2(np.e)) / 16.0
B_ADD = 56.0 - 0.344

_CACHE = {}


def build_module(
    reps: int = 1,
    st_bufs: int = 2,
    op_bufs: int = 4,
    ep_bufs: int = 6,
    pp_bufs: int = 4,
    dve_pairs: tuple = (1, 4, 6, 9, 11, 14),
    act_copies: bool = True,
    act_scales: bool = True,
):
    nc = bacc.Bacc("TRN2", target_bir_lowering=False)
    jp_d = nc.dram_tensor("jp", [CE, HW], F32R, kind="ExternalInput")
    jg_d = nc.dram_tensor("jg", [CE, HW], F32R, kind="ExternalInput")
    wq_d = nc.dram_tensor("wq", [CE, D], F32R, kind="ExternalInput")
    wk_d = nc.dram_tensor("wk", [CE, D], F32R, kind="ExternalInput")
    wv_d = nc.dram_tensor("wv", [CE, D], F32R, kind="ExternalInput")
    ones_d = nc.dram_tensor("ones", [128, N_KC, 2], F8, kind="ExternalInput")
    out_d = nc.dram_tensor("out", [HW, D], F32, kind="ExternalOutput")

    dve_set = set(dve_pairs)

    with tile.TileContext(nc) as tc:
        with tc.tile_pool(name="const", bufs=1) as const:
            jp_t = const.tile([CE, HW], F32R, tag="jp")
            jg_t = const.tile([CE, HW], F32R, tag="jg")
            wq_t = const.tile([CE, D], F32R, tag="wq")
            wk_t = const.tile([CE, D], F32R, tag="wk")
            wv_t = const.tile([CE, D], F32R, tag="wv")
            qt_b = [
                const.tile([128, 2, QB], F8, tag=f"qt{g}", name=f"qt_{g}")
                for g in range(N_QB)
            ]
            # K tiles in SwInterleave layout: [128, 4 chunks, 256] where
            # column 2*(127-q)+dh of chunk j holds K[dh, 128j+q]
            kt_g = [
                const.tile([128, 4, 256], F8, tag=f"kt{g}", name=f"kt_{g}")
                for g in range(N_QB)
            ]
            vt_g = [
                const.tile([128, 4, DV], F8, tag=f"vt{g}", name=f"vt_{g}")
                for g in range(N_QB)
            ]

            nc.sync.dma_start(wq_t[:], wq_d[:])
            nc.sync.dma_start(wk_t[:], wk_d[:])
            nc.sync.dma_start(wv_t[:], wv_d[:])
            for g in range(N_QB):
                hs = slice(g * QB, (g + 1) * QB)
                nc.sync.dma_start(jg_t[:, hs], jg_d[:, hs])
                nc.sync.dma_start(jp_t[:, hs], jp_d[:, hs])
                nc.sync.dma_start(vt_g[g][:, :, D:DV], ones_d[:, 4 * g : 4 * g + 4, :])

            def copy_f8(eng_is_act, dst, src):
                if eng_is_act and act_copies:
                    nc.scalar.activation(
                        dst, src, mybir.ActivationFunctionType.Copy
                    )
                else:
                    nc.vector.tensor_copy(dst, src)

            for _rep in range(reps):
                # ---- projections (f32r) ----
                with tc.tile_pool(name="pp", bufs=pp_bufs, space="PSUM") as pp:

                    def proj_q(g):
                        hs = slice(g * QB, (g + 1) * QB)
                        for dh in range(2):
                            ds = slice(dh * 128, (dh + 1) * 128)
                            psq = pp.tile([128, QB], F32, tag="proj")
                            nc.tensor.matmul(psq[:], wq_t[:, ds], jp_t[:, hs])
                            copy_f8(dh == 1, qt_b[g][:, dh, :], psq[:])

                    proj_q(0)
                    for g in range(N_QB):
                        hs = slice(g * QB, (g + 1) * QB)
                        for dh in range(2):
                            ds = slice(dh * 128, (dh + 1) * 128)
                            psk = pp.tile([128, QB], F32, tag="proj")
                            nc.tensor.matmul(psk[:], wk_t[:, ds], jg_t[:, hs])
                            dst = dataclasses.replace(
                                kt_g[g][:],
                                ap=[[1024, 128], [256, 4], [-2, 128]],
                                offset=kt_g[g][:].offset + 254 + dh,
                            )
                            nc.vector.tensor_copy(dst, psk[:])
                        for j in range(4):
                            ck = 4 * g + j
                            ks = slice(ck * 128, (ck + 1) * 128)
                            psv = pp.tile([128, D], F32, tag="projv")
                            nc.tensor.matmul(psv[:], jg_t[:, ks], wv_t[:])
                            copy_f8(j % 2 == 0, vt_g[g][:, j, :D], psv[:])
                    for g in range(1, N_QB):
                        proj_q(g)

                # ---- attention ----
                with (
                    tc.tile_pool(name="stp", bufs=st_bufs, space="PSUM") as stp,
                    tc.tile_pool(name="op", bufs=op_bufs, space="PSUM") as op,
                    tc.tile_pool(name="ep", bufs=ep_bufs) as ep,
                    tc.tile_pool(name="outp", bufs=3) as outp,
                    tc.tile_pool(name="lp", bufs=4) as lp,
                ):
                    for qb in range(N_QB):
                        o_ps = [
                            op.tile([128, DV], F32, tag="o", name=f"o_{qb}_{i}")
                            for i in range(4)
                        ]
                        st_t = [None] * NPAIR
                        et_t = [None] * NPAIR

                        def s_pair(p):
                            st2 = stp.tile([128, 2, QB], F32, tag="st")
                            st_t[p] = st2
                            for c in range(2):
                                ck = 2 * p + c
                                g, j = ck // 4, ck % 4
                                nc.tensor.matmul(
                                    st2[:, c, :],
                                    kt_g[g][:, j, :],
                                    qt_b[qb][:],
                                    start=True,
                                    stop=True,
                                    perf_mode=SWI,
                                )

                        def exp_pair(p):
                            et2 = ep.tile([128, 2, QB], F8, tag="e")
                            et_t[p] = et2
                            if p in dve_set:
                                nc.vector.tensor_scalar(
                                    et2[:].bitcast(U8),
                                    st_t[p][:, :, :],
                                    A_MUL,
                                    B_ADD,
                                    mybir.AluOpType.mult,
                                    mybir.AluOpType.add,
                                )
                            else:
                                nc.scalar.activation(
                                    et2[:, :, :],
                                    st_t[p][:, :, :],
                                    mybir.ActivationFunctionType.Exp,
                                    scale=1.0 / 16.0,
                                )

                        def pv_pair(p):
                            g, jj = p // 2, 2 * (p % 2)
                            et2 = et_t[p]
                            for i in range(4):
                                nc.tensor.matmul(
                                    o_ps[i][:],
                                    et2[:, :, i * 128 : (i + 1) * 128],
                                    vt_g[g][:, jj : jj + 2, :],
                                    start=(p == 0),
                                    stop=(p == NPAIR - 1),
                                    perf_mode=DR,
                                )

                        # software pipeline: issue S(p+1) before PV(p) so the
                        # PE never sits behind exp(p) with S work available
                        s_pair(0)
                        for p in range(NPAIR):
                            exp_pair(p)
                            if p + 1 < NPAIR:
                                s_pair(p + 1)
                            pv_pair(p)

                        for qsub in range(4):
                            row = qb * 4 + qsub
                            linv = lp.tile([128, 1], F32, tag="l")
                            nc.vector.reciprocal(linv[:], o_ps[qsub][:, D : D + 1])
                            ot = outp.tile([128, D], F32, tag="ot")
                            if qsub % 2 == 0 and act_scales:
                                nc.scalar.activation(
                                    ot[:],
                                    o_ps[qsub][:, :D],
                                    mybir.ActivationFunctionType.Copy,
                                    scale=linv[:],
                                )
                            else:
                                nc.vector.tensor_scalar_mul(
                                    ot[:], o_ps[qsub][:, :D], linv[:]
                                )
                            nc.sync.dma_start(
                                out_d[row * 128 : (row + 1) * 128, :], ot[:]
                            )

    nc.compile()
    return nc


def _get_module(reps: int = 1, **kw):
    key = (reps, tuple(sorted(kw.items())))
    if key not in _CACHE:
        _CACHE[key] = build_module(reps, **kw)
    return _CACHE[key]


_ROW1 = np.ones((1, HW), np.float32)


def _prep_in_maps(inputs, **_):
    import ml_dtypes

    f8 = ml_dtypes.float8_e4m3
    jp = np.asarray(inputs["Jp_embedding"], np.float32).reshape(B, C, HW)
    jg = np.asarray(inputs["Jg_embedding"], np.float32).reshape(B, C, HW)
    wq = np.concatenate(
        [
            np.asarray(inputs["Wq"], np.float32).T,
            np.asarray(inputs["bq"], np.float32)[None, :],
        ],
        0,
    )
    wk = np.concatenate(
        [
            np.asarray(inputs["Wk"], np.float32).T,
            np.asarray(inputs["bk"], np.float32)[None, :],
        ],
        0,
    )
    wv = np.concatenate(
        [
            np.asarray(inputs["Wv"], np.float32).T,
            np.asarray(inputs["bv"], np.float32)[None, :],
        ],
        0,
    )
    ones = np.ones((128, N_KC, 2), f8)
    return [
        {
            "jp": np.concatenate([jp[b], _ROW1], 0),
            "jg": np.concatenate([jg[b], _ROW1], 0),
            "wq": wq,
            "wk": wk,
            "wv": wv,
            "ones": ones,
        }
        for b in range(B)
    ]


def kernel(**inputs):
    nc = _get_module()
    in_maps = _prep_in_maps(inputs)
    # The first execution of a freshly loaded NEFF can race DMA/engine
    # warm-up (observed on HW; CoreSim-clean). Run once to warm up, then
    # return the second execution's results.
    run_bass_kernel_spmd(nc, in_maps, core_ids=list(range(N_CORES)))
    res = run_bass_kernel_spmd(nc, in_maps, core_ids=list(range(N_CORES)))
    return np.stack(
        [res.results[b]["out"].reshape(D, H, W) for b in range(B)], axis=0
    )
